# revision 17
# baseline (speedup 1.0000x reference)
"""BitNet Llama attention (B=2, S=2048, H=4096, 32 q-heads / 8 kv-heads, GQA),
distributed over 8 Trainium2 NeuronCores.

v3 sharding: every core receives the FULL hidden_states and quantizes all
4096 tokens in 8 streamed 512-token chunks; each core computes projections
only for ITS OWN heads (q-heads 4c..4c+3 + kv-head c) over all tokens --
per-core weight column slices arrive as input data, so the instruction
stream is identical on every core (SPMD) while the q/k/v AllToAll
collectives disappear entirely (measured ~100 us fixed cost each on this
fabric).  Attention is head-sharded (full causal triangle).  The o-path is
unchanged from v2: per-batch AllReduce(max) for o-proj activation scales
(partition-major layout, AllReduce is elementwise so layout is free),
quantize + batched transpose + int8 AllToAll back to token shards, BitLinear
o_proj over the core's own 512 tokens, host concat of row slices.

BitLinear exactness: weights are ternarized on host and shipped as bf16
{-1,0,1}; activations are quantized on-chip to the int8 grid (magic-number
round-half-even on DVE) and stored as bf16 integers; bf16 x bf16 matmuls
accumulate exact integers in fp32 PSUM; per-token dequant scales applied
afterwards.  amax reductions run on GpSimd (max is rounding-insensitive) to
keep DVE off the critical path.
"""

import math
import os
import sys
from contextlib import ExitStack

import numpy as np
import ml_dtypes

for _p in ("/opt/trn_rl_repo", os.path.expanduser("~/.axon_site/_ro/trn_rl_repo")):
    if os.path.isdir(_p) and _p not in sys.path:
        sys.path.insert(0, _p)

import concourse.bass as bass
import concourse.mybir as mybir
import concourse.tile as tile
from concourse import bacc

P = 128
H = 4096
DHEAD = 128
NH = 32
NKV = 8
NCORES = 8
MAGIC = 12582912.0  # 1.5 * 2**23: fp32 round-half-even via add/sub
LN2 = float(math.log(2.0))
INV_SQRT_D = float(np.float32(1.0) / np.float32(np.sqrt(np.float32(DHEAD))))
INV127 = float(np.float32(1.0) / np.float32(127.0))

F32 = mybir.dt.float32
BF16 = mybir.dt.bfloat16
I8 = mybir.dt.int8
MULT = mybir.AluOpType.mult
ADD = mybir.AluOpType.add
SUB = mybir.AluOpType.subtract
MAXOP = mybir.AluOpType.max


def build_program(S=2048, B=2, collectives=True):
    """One SPMD program; per-core behavior differs only through input data."""
    T_GLOB = B * S                      # global tokens
    T_OWN = T_GLOB // NCORES            # tokens owned per core
    NT = T_OWN // P                     # own token tiles (4 at S=2048)
    QTB = S // P                        # q tiles per batch (16)
    QT_ALL = B * QTB                    # global token tiles (32)
    HT = H // P                         # hidden tiles (32)
    GF = H // NCORES                    # q-features per head group (512)
    T_CH = 256                          # streamed chunk tokens (SBUF budget)
    NCH = T_GLOB // T_CH                # streamed token chunks (16)
    NTC = T_CH // P                     # token tiles per chunk (2)

    nc = bacc.Bacc(
        "TRN2", target_bir_lowering=False, debug=False, num_devices=NCORES
    )
    groups = [list(range(NCORES))]

    x_full = nc.dram_tensor("x_full", [T_GLOB, H], F32, kind="ExternalInput")
    wq_own = nc.dram_tensor("wq_own", [H, GF], BF16, kind="ExternalInput")
    wk_own = nc.dram_tensor("wk_own", [H, DHEAD], BF16, kind="ExternalInput")
    wv_own = nc.dram_tensor("wv_own", [H, DHEAD], BF16, kind="ExternalInput")
    woT = nc.dram_tensor("woT", [H, H], BF16, kind="ExternalInput")
    scal = nc.dram_tensor("scal", [P, 8], F32, kind="ExternalInput")
    cmaskT = nc.dram_tensor("cmaskT", [P, 4 * P], BF16, kind="ExternalInput")
    smask = nc.dram_tensor("smask", [P, NT * QT_ALL], F32, kind="ExternalInput")
    out_own = nc.dram_tensor("out_own", [T_OWN, H], F32, kind="ExternalOutput")

    with tile.TileContext(nc) as tc, ExitStack() as ctx:
        dram = ctx.enter_context(tc.tile_pool(name="dram", bufs=1, space="DRAM"))
        const = ctx.enter_context(tc.tile_pool(name="const", bufs=1))

        # partition-major amax partials: [p, qtile] per batch half
        pamax_d = dram.tile([B, P, QTB], F32)
        amax_all_d = dram.tile([B, P, QTB], F32)
        # o int8 chunks: [chunk r][ftile][qt-in-chunk][feat128][tok128]
        xo8_in = dram.tile([NCORES, 4, NT, P, P], I8, allow_tmpbuf=True)
        xo8_out = dram.tile([NCORES, 4, NT, P, P], I8, allow_tmpbuf=True)

        cmask_sb = const.tile([P, 4 * P], BF16)
        nc.sync.dma_start(cmask_sb[:], cmaskT[:, :])
        scal_sb = const.tile([P, 8], F32)
        nc.sync.dma_start(scal_sb[:], scal[:, :])
        smask_sb = const.tile([P, NT * QT_ALL], F32)
        nc.sync.dma_start(smask_sb[:], smask[:, :])

        # my heads' weight slabs, resident for the whole projection stream
        wq_sb = const.tile([P, HT, GF], BF16)
        nc.sync.dma_start(wq_sb[:], wq_own.rearrange("(hi p) o -> p hi o", p=P))
        wk_sb = const.tile([P, HT, DHEAD], BF16)
        nc.sync.dma_start(wk_sb[:], wk_own.rearrange("(hi p) o -> p hi o", p=P))
        wv_sb = const.tile([P, HT, DHEAD], BF16)
        nc.sync.dma_start(wv_sb[:], wv_own.rearrange("(hi p) o -> p hi o", p=P))

        # own-token amax columns (filled after attention; used by o_proj)
        amx_cm = tc.tile_pool(name="amx", bufs=1)
        amx = amx_cm.__enter__()
        amax_own_cols = amx.tile([P, NT], F32)

        # persistent attention operands, filled by the projection stream
        pat_cm = tc.tile_pool(name="pat", bufs=1)
        pat = pat_cm.__enter__()
        qT_grp = pat.tile([P, 4, T_GLOB], BF16)
        kT_full = pat.tile([P, T_GLOB], BF16)
        v_full = pat.tile([P, QT_ALL, 144], BF16)  # 288B row stride: xbar-aligned
        nc.vector.memset(v_full[:], 1.0)  # column 128 = denominator ones

        # ---- streamed projection loop: quantize + K/V/Q for own heads ----
        with nc.named_scope("proj_stream"), \
             tc.tile_pool(name="qwork", bufs=2) as qwork, \
             tc.tile_pool(name="xch", bufs=2) as xch, \
             tc.tile_pool(name="bw", bufs=2) as bw, \
             tc.tile_pool(name="pev", bufs=3) as pev, \
             tc.tile_pool(name="psp", bufs=4, space="PSUM") as psp, \
             tc.tile_pool(name="psb", bufs=2, space="PSUM") as psb:
            for tch in range(NCH):
                xqT = xch.tile([P, HT, T_CH], BF16, tag="xqT")
                dq_cols = bw.tile([P, NTC], F32, tag="dqc")
                for ti in range(NTC):
                    gt = tch * NTC + ti
                    x_t = qwork.tile([P, H], F32, tag="x")
                    nc.sync.dma_start(x_t[:], x_full[gt * P:(gt + 1) * P, :])
                    amax = qwork.tile([P, 1], F32, tag="amax")
                    nc.vector.tensor_reduce(
                        amax[:], x_t[:], mybir.AxisListType.X, MAXOP,
                        apply_absolute_value=True,
                    )
                    amax_c = qwork.tile([P, 1], F32, tag="amaxc")
                    nc.vector.tensor_scalar(amax_c[:], amax[:], 1e-5, None, MAXOP)
                    inv = qwork.tile([P, 1], F32, tag="inv")
                    nc.vector.reciprocal(inv[:], amax_c[:])
                    a_col = qwork.tile([P, 1], F32, tag="acol")
                    nc.vector.tensor_scalar(a_col[:], inv[:], 127.0, None, MULT)
                    nc.vector.tensor_scalar(
                        dq_cols[:, ti:ti + 1], amax_c[:], INV127, None, MULT
                    )
                    nc.vector.tensor_scalar(
                        x_t[:], x_t[:], a_col[:], MAGIC, MULT, ADD
                    )
                    xq = qwork.tile([P, H], BF16, tag="xq", bufs=1)
                    nc.vector.tensor_scalar(xq[:], x_t[:], MAGIC, None, SUB)
                    # one batched transpose: out[p, hi, t] = xq[t, hi*128+p]
                    nc.sync.dma_start_transpose(
                        xqT[:, :, ti * P:(ti + 1) * P], xq[:, :]
                    )

                # per-token dequant rows broadcast across partitions
                dq_row = bw.tile([1, T_CH], F32, tag="dqr")
                for ti in range(NTC):
                    nc.sync.dma_start(
                        dq_row[0:1, ti * P:(ti + 1) * P], dq_cols[:, ti:ti + 1]
                    )
                ones_row = bw.tile([1, P], F32, tag="ones")
                nc.vector.memset(ones_row[:], 1.0)
                srow_q = bw.tile([1, T_CH], F32, tag="sq")
                nc.vector.tensor_scalar(
                    srow_q[:], dq_row[:], scal_sb[0:1, 0:1], INV_SQRT_D,
                    MULT, MULT,
                )
                srow_k = bw.tile([1, T_CH], F32, tag="sk")
                nc.vector.tensor_scalar(
                    srow_k[:], dq_row[:], scal_sb[0:1, 1:2], None, MULT
                )
                srow_v = bw.tile([1, T_CH], F32, tag="sv")
                nc.vector.tensor_scalar(
                    srow_v[:], dq_row[:], scal_sb[0:1, 2:3], None, MULT
                )
                bcasts = {}
                for nm, srow in (("q", srow_q), ("k", srow_k), ("v", srow_v)):
                    ps = psb.tile([P, T_CH], F32, tag="b")
                    nc.tensor.matmul(
                        ps[:], ones_row[:], srow[:], start=True, stop=True
                    )
                    dst = bw.tile([P, T_CH], F32, tag=f"bc{nm}")
                    nc.vector.tensor_copy(dst[:], ps[:])
                    bcasts[nm] = dst

                # K projection for my kv-head over this chunk
                ps = psp.tile([P, T_CH], F32, tag="p")
                for hi in range(HT):
                    nc.tensor.matmul(
                        ps[:], wk_sb[:, hi, :], xqT[:, hi, :],
                        start=(hi == 0), stop=(hi == HT - 1),
                    )
                nc.vector.tensor_tensor(
                    kT_full[:, tch * T_CH:(tch + 1) * T_CH], ps[:],
                    bcasts["k"][:], MULT,
                )
                # V projection (transposed orientation), then xbar to [tok, dh]
                ps = psp.tile([P, T_CH], F32, tag="p")
                for hi in range(HT):
                    nc.tensor.matmul(
                        ps[:], wv_sb[:, hi, :], xqT[:, hi, :],
                        start=(hi == 0), stop=(hi == HT - 1),
                    )
                vT_c = pev.tile([P, T_CH], BF16, tag="vT")
                nc.vector.tensor_tensor(vT_c[:], ps[:], bcasts["v"][:], MULT)
                nc.sync.dma_start_transpose(
                    v_full[:, tch * NTC:(tch + 1) * NTC, 0:P], vT_c[:, :]
                )
                # Q projection for my 4 heads over this chunk
                for hl in range(4):
                    ps = psp.tile([P, T_CH], F32, tag="p")
                    for hi in range(HT):
                        nc.tensor.matmul(
                            ps[:], wq_sb[:, hi, hl * P:(hl + 1) * P],
                            xqT[:, hi, :],
                            start=(hi == 0), stop=(hi == HT - 1),
                        )
                    nc.vector.tensor_tensor(
                        qT_grp[:, hl, tch * T_CH:(tch + 1) * T_CH], ps[:],
                        bcasts["q"][:], MULT,
                    )

        # ---- attention (full causal triangle, 4 heads), per batch ----
        pos_cm = tc.tile_pool(name="pos", bufs=1)
        pos = pos_cm.__enter__()
        o_slice = [pos.tile([P, QTB, GF], BF16, name=f"osl{_b}") for _b in range(B)]
        pamax_sb = [pos.tile([P, QTB], F32, name=f"pam{_b}") for _b in range(B)]
        a_all_sb = [pos.tile([P, QTB], F32, name=f"aal{_b}") for _b in range(B)]
        att_cm = tc.tile_pool(name="att", bufs=2)
        att = att_cm.__enter__()
        pss_cm = tc.tile_pool(name="pss", bufs=2, space="PSUM")
        pss = pss_cm.__enter__()
        pso_cm = tc.tile_pool(name="pso", bufs=1, space="PSUM")
        pso = pso_cm.__enter__()
        oq_cm = tc.tile_pool(name="oq", bufs=6)
        oq = oq_cm.__enter__()

        for b in range(B):
            with nc.named_scope(f"attn_b{b}"):
                for qb in range(QTB):
                    qt = b * QTB + qb
                    po = [pso.tile([P, 132], F32, name=f"po{_h}") for _h in range(4)]
                    pt_all = att.tile([P, QTB, 4 * P], BF16, tag="pt", bufs=2)
                    for kk in range(0, qb + 1, 2):
                        nk = min(2, qb + 1 - kk)
                        ps = pss.tile([P, 1024], F32, tag="s")
                        for t in range(nk):
                            kt = b * QTB + kk + t
                            nc.tensor.matmul(
                                ps[:, t * 512:(t + 1) * 512],
                                kT_full[:, kt * P:(kt + 1) * P],
                                qT_grp[:, :, qt * P:(qt + 1) * P],
                                start=True, stop=True,
                            )
                        nc.scalar.activation(
                            pt_all[:, kk:kk + nk, :], ps[:, 0:nk * 512],
                            mybir.ActivationFunctionType.Exp, scale=LN2,
                        )
                        if kk + nk - 1 == qb:
                            nc.vector.tensor_tensor(
                                pt_all[:, qb, :], pt_all[:, qb, :],
                                cmask_sb[:], MULT,
                            )
                        for t in range(nk):
                            j = kk + t
                            kt = b * QTB + j
                            for hl in range(4):
                                nc.tensor.matmul(
                                    po[hl][:, 0:129],
                                    pt_all[:, j, hl * P:(hl + 1) * P],
                                    v_full[:, kt, 0:129],
                                    start=(j == 0), stop=(j == qb),
                                )
                    for hl in range(4):
                        den = att.tile([P, 1], F32, tag="den")
                        nc.vector.reciprocal(den[:], po[hl][:, 128:129])
                        nc.vector.tensor_scalar(
                            o_slice[b][:, qb, hl * P:(hl + 1) * P],
                            po[hl][:, 0:P], den[:], None, MULT,
                        )
                    nc.vector.tensor_reduce(
                        pamax_sb[b][:, qb:qb + 1], o_slice[b][:, qb, :],
                        mybir.AxisListType.X, MAXOP, apply_absolute_value=True,
                    )

            # ---- per-batch: global amax, quantize o, transpose, int8 ----
            with nc.named_scope(f"ochain_b{b}"):
                nc.gpsimd.dma_start(pamax_d[b, :, :], pamax_sb[b][:, :])
                if collectives:
                    nc.gpsimd.collective_compute(
                        "AllReduce", MAXOP, replica_groups=groups,
                        ins=[pamax_d[b, :, :].opt()],
                        outs=[amax_all_d[b, :, :].opt()],
                    )
                else:
                    nc.sync.dma_start(amax_all_d[b, :, :], pamax_d[b, :, :])
                nc.gpsimd.dma_start(a_all_sb[b][:, :], amax_all_d[b, :, :])
                for qb in range(QTB):
                    qt = b * QTB + qb
                    am_c = oq.tile([P, 1], F32, tag="amc")
                    nc.vector.tensor_scalar(
                        am_c[:], a_all_sb[b][:, qb:qb + 1], 1e-5, None, MAXOP
                    )
                    inv = oq.tile([P, 1], F32, tag="oinv")
                    nc.vector.reciprocal(inv[:], am_c[:])
                    a_col = oq.tile([P, 1], F32, tag="oacol")
                    nc.vector.tensor_scalar(a_col[:], inv[:], 127.0, None, MULT)
                    xr = oq.tile([P, GF], F32, tag="oxr")
                    nc.vector.tensor_scalar(
                        xr[:], o_slice[b][:, qb, :], a_col[:], MAGIC, MULT, ADD
                    )
                    xqo = oq.tile([P, GF], BF16, tag="oxq")
                    nc.vector.tensor_scalar(xqo[:], xr[:], MAGIC, None, SUB)
                    # transpose to [feat128, ftile, tok128], then int8-cast
                    oT = oq.tile([P, 4, P], BF16, tag="oT")
                    nc.sync.dma_start_transpose(oT[:, :, :], xqo[:, :])
                    o8 = oq.tile([P, 4, P], I8, tag="o8")
                    nc.vector.tensor_copy(o8[:], oT[:])
                    nc.gpsimd.dma_start(
                        xo8_in[qt // NT, :, qt % NT, :, :].rearrange(
                            "f p t -> p f t"
                        ),
                        o8[:, :, :],
                    )

        if collectives:
            nc.gpsimd.collective_compute(
                "AllToAll", mybir.AluOpType.bypass, replica_groups=groups,
                ins=[xo8_in[:, :, :, :, :].opt()],
                outs=[xo8_out[:, :, :, :, :].opt()],
            )
        else:
            nc.sync.dma_start(
                xo8_out[:, :, :, :, :], xo8_in[:, :, :, :, :]
            )

        # own-token amax via one-hot select from the all-token amax table
        # (own tokens live in exactly one batch; the other batch's mask is 0,
        # and amax values are >= 0, so combining the two via MAX is exact)
        with tc.tile_pool(name="selw", bufs=2) as selw:
            for ti in range(NT):
                red = []
                for b in range(B):
                    st = selw.tile([P, QTB], F32, tag="st")
                    nc.vector.tensor_tensor(
                        st[:], a_all_sb[b][:, :],
                        smask_sb[:, ti * QT_ALL + b * QTB:
                                 ti * QT_ALL + (b + 1) * QTB],
                        MULT,
                    )
                    r = selw.tile([P, 1], F32, tag="r")
                    nc.vector.tensor_reduce(
                        r[:], st[:], mybir.AxisListType.X, MAXOP,
                        apply_absolute_value=False,
                    )
                    red.append(r)
                nc.vector.tensor_tensor(
                    amax_own_cols[:, ti:ti + 1], red[0][:], red[1][:], MAXOP
                )

        oq_cm.__exit__(None, None, None)
        pso_cm.__exit__(None, None, None)
        pss_cm.__exit__(None, None, None)
        att_cm.__exit__(None, None, None)
        pos_cm.__exit__(None, None, None)
        pat_cm.__exit__(None, None, None)

        # ---- assemble o^T (int8 -> bf16 cast during SWDGE DMA) ----
        pxo_cm = tc.tile_pool(name="pxo", bufs=1)
        pxo = pxo_cm.__enter__()
        xoqT = pxo.tile([P, HT, T_OWN], BF16)
        with nc.named_scope("F_assemble"):
            for s in range(NCORES):
                nc.gpsimd.dma_start(
                    xoqT[:, 4 * s:4 * s + 4, :].rearrange(
                        "p f (q t) -> p (f q) t", q=NT
                    ),
                    xo8_out[s, :, :, :, :].rearrange("f q p t -> p (f q) t"),
                )

        # ---- o_proj (token-sharded, full output features) ----
        with nc.named_scope("G_oproj"), \
             tc.tile_pool(name="gw", bufs=2) as gw, \
             tc.tile_pool(name="gev", bufs=3) as gev, \
             tc.tile_pool(name="psg", bufs=3, space="PSUM") as psg:
            woT_r = woT.rearrange("(hi p) o -> p hi o", p=P)
            dqo_cols = gev.tile([P, NT], F32, tag="dqo")
            tmpc = gev.tile([P, NT], F32, tag="tc")
            nc.vector.tensor_scalar(tmpc[:], amax_own_cols[:], 1e-5, None, MAXOP)
            nc.vector.tensor_scalar(
                dqo_cols[:], tmpc[:], scal_sb[:, 3:4], INV127, MULT, MULT
            )
            for nj in range(H // 512):
                wsl = gw.tile([P, HT, 512], BF16, tag="wo")
                nc.sync.dma_start(wsl[:], woT_r[:, :, nj * 512:(nj + 1) * 512])
                for ti in range(NT):
                    ps = psg.tile([P, 512], F32, tag="g")
                    for hi in range(HT):
                        nc.tensor.matmul(
                            ps[:], xoqT[:, hi, ti * P:(ti + 1) * P], wsl[:, hi, :],
                            start=(hi == 0), stop=(hi == HT - 1),
                        )
                    ev = gev.tile([P, 512], F32, tag="ge")
                    nc.vector.tensor_scalar(
                        ev[:], ps[:], dqo_cols[:, ti:ti + 1], None, MULT
                    )
                    nc.scalar.dma_start(
                        out_own[ti * P:(ti + 1) * P, nj * 512:(nj + 1) * 512], ev[:]
                    )

        pxo_cm.__exit__(None, None, None)
        amx_cm.__exit__(None, None, None)

    nc.compile()
    return nc


def _ternarize(W):
    ws = np.float32(max(np.mean(np.abs(W), dtype=np.float32), np.float32(1e-5)))
    t = np.clip(np.round(W / ws), -1.0, 1.0).astype(np.float32)
    return t, ws


def prepare_inputs(hidden_states, Wq, Wk, Wv, Wo, S=2048, B=2):
    bf16 = ml_dtypes.bfloat16
    T_GLOB = B * S
    T_OWN = T_GLOB // NCORES
    NT = T_OWN // P
    QTB = S // P
    QT_ALL = B * QTB
    GF = H // NCORES
    x = np.ascontiguousarray(
        np.asarray(hidden_states, dtype=np.float32).reshape(T_GLOB, H)
    )
    tq, wqs = _ternarize(np.asarray(Wq, dtype=np.float32))
    tk, wks = _ternarize(np.asarray(Wk, dtype=np.float32))
    tv, wvs = _ternarize(np.asarray(Wv, dtype=np.float32))
    to, wos = _ternarize(np.asarray(Wo, dtype=np.float32))
    wqT = np.ascontiguousarray(tq.T).astype(bf16)
    wkT = np.ascontiguousarray(tk.T).astype(bf16)
    wvT = np.ascontiguousarray(tv.T).astype(bf16)
    woT = np.ascontiguousarray(to.T).astype(bf16)
    scal = np.zeros((P, 8), np.float32)
    scal[:, 0] = wqs
    scal[:, 1] = wks
    scal[:, 2] = wvs
    scal[:, 3] = wos
    kk, qq = np.meshgrid(np.arange(P), np.arange(P), indexing="ij")
    cmaskT = np.tile((kk <= qq).astype(np.float32).astype(bf16), (1, 4))
    shared = dict(x_full=x, woT=woT, scal=scal, cmaskT=cmaskT)
    maps = []
    for c in range(NCORES):
        sm = np.zeros((P, NT * QT_ALL), np.float32)
        for ti in range(NT):
            sm[:, ti * QT_ALL + 4 * c + ti] = 1.0
        maps.append(dict(
            wq_own=np.ascontiguousarray(wqT[:, c * GF:(c + 1) * GF]),
            wk_own=np.ascontiguousarray(wkT[:, c * DHEAD:(c + 1) * DHEAD]),
            wv_own=np.ascontiguousarray(wvT[:, c * DHEAD:(c + 1) * DHEAD]),
            smask=sm, **shared,
        ))
    return maps


_PROGRAM_CACHE = {}


def kernel(hidden_states, attention_mask, Wq, Wk, Wv, Wo):
    from concourse.bass_utils import run_bass_kernel_spmd

    B, S, _ = hidden_states.shape
    key = (B, S)
    if key not in _PROGRAM_CACHE:
        _PROGRAM_CACHE[key] = build_program(S=S, B=B)
    nc = _PROGRAM_CACHE[key]
    in_maps = prepare_inputs(hidden_states, Wq, Wk, Wv, Wo, S=S, B=B)
    res = run_bass_kernel_spmd(
        nc, in_maps, core_ids=list(range(NCORES)),
        trace=bool(int(os.environ.get("KERNEL_TRACE", "0"))),
    )
    out = np.concatenate([r["out_own"] for r in res.results], axis=0)
    kernel.last_results = res
    return np.ascontiguousarray(out.reshape(B, S, H)).astype(np.float32)
